# revision 1
# baseline (speedup 1.0000x reference)
"""Trainium2 Bass kernel for nn_Attention_40475771798025.

Full attention layer: QKV projection + RoPE + GQA causal attention + output
projection. B=2, S=2048, D=4096, H=32 q-heads, KV=8 kv-heads, HD=128.

Sharding: head-parallel tensor parallelism across 8 cores. Core g owns kv-head
g (its 4 q-heads, 1 k-head, 1 v-head) for both batches. Weights are
pre-transposed on the host so the device kernel runs pure [K=128]-contraction
matmuls in float32r (full PE rate, ~tf32 precision). The output projection
produces per-core partial sums of the full [T, D] output, summed on the host.

Device kernel per core, per batch:
  A: qkv^T = wqkvT^T @ x^T accumulated over graded D-chunk passes (SBUF
     accumulation, paired-tile PSUM evacuation; first passes are small so the
     pipeline fills fast), RoPE applied in place via a constant
     rotation-matrix matmul + GPSIMD/DVE combine (k-head and q0 at the end of
     A; q1-3 under phase B's h-blocks), V transposed to natural layout with
     PE transposes.
  B: per q-head, per 512-wide q-tile: scores^T tiles [128k, 512q] on PE,
     causal mask as additive -1e9 bias on the diagonal-straddle tiles (DVE),
     exp on ACT (scale=1/sqrt(128), no max-subtraction needed at these scales),
     E@V and all-ones denominator matmuls accumulate in PSUM over k-tiles two
     iterations behind the scores matmul (hides the mask+exp latency),
     normalize with DVE reciprocal+mul, written back into the q slot of the
     qkv accumulator (its reads are complete by then).
  C: partial out = attT^T @ woT (streamed in quarters), PSUM->SBUF->DRAM.

Streaming pools (x/w chunks, wo quarters) live at top level so DMA prefetch
crosses batch and phase boundaries.
"""
import sys
sys.path.insert(0, "/opt/trn_rl_repo")
import numpy as np

B, S, D = 2, 2048, 4096
H, KV, HD = 32, 8, 128
REP = H // KV            # 4 q-heads per core
T = B * S                # 4096 flattened tokens
NCORES = 8
P = 128
QTW, KTW = 512, 128      # q-tile width (psum free dim), k-tile width
MQKV = REP + 2           # 6 m-tiles of 128: q0..q3, k, v
SCALE = 1.0 / float(np.sqrt(HD))

_nc = None


def _build_nc(reps=1):
    import concourse.bacc as bacc
    import concourse.mybir as mybir
    import concourse.tile as tile
    from contextlib import ExitStack

    F32 = mybir.dt.float32
    F32R = mybir.dt.float32r

    nc = bacc.Bacc("TRN2")
    xT_d = nc.dram_tensor("xT", (D, T), F32, kind="ExternalInput")
    wqkvT_d = nc.dram_tensor("wqkvT", (D, MQKV * P), F32, kind="ExternalInput")
    woT_d = nc.dram_tensor("woT", (REP * P, D), F32, kind="ExternalInput")
    cdup_d = nc.dram_tensor("cdup", (P, T), F32, kind="ExternalInput")
    sdup_d = nc.dram_tensor("sdup", (P, T), F32, kind="ExternalInput")
    pt_d = nc.dram_tensor("pt", (P, P), F32, kind="ExternalInput")
    ones_d = nc.dram_tensor("ones", (P, P), F32, kind="ExternalInput")
    ident_d = nc.dram_tensor("ident", (P, P), F32, kind="ExternalInput")
    maskb_d = nc.dram_tensor("maskb", (P, QTW // KTW, QTW), F32,
                             kind="ExternalInput")
    out_d = nc.dram_tensor("out", (T, D), F32, kind="ExternalOutput")

    NT = S // QTW            # 4 q-tiles per batch
    NKT = S // KTW           # 16 k-tiles per batch
    # D contraction pass sizes in 128-chunks; first passes small so the
    # post-boundary DMA exposure is tiny (PE starts after ~1.4MB, not 5.5MB)
    CSIZES = [1, 2, 3, 4, 4, 4, 4, 4, 3, 3]
    assert sum(CSIZES) == D // P
    NWQ = 4                  # wo slices
    NQ = D // NWQ            # 1024 output cols per wo slice
    M_ORDER = [REP, 0, REP + 1, 1, 2, 3]   # k, q0, v, q1-3
    PIPE = 4                 # phase-B EV pipeline depth

    with tile.TileContext(nc) as tc, ExitStack() as top:
        persist = top.enter_context(tc.tile_pool(name="persist", bufs=1))
        accp = top.enter_context(tc.tile_pool(name="acc", bufs=1))
        vnp = top.enter_context(tc.tile_pool(name="vnat", bufs=1))
        csp = top.enter_context(tc.tile_pool(name="cs", bufs=2))
        tmpp = top.enter_context(tc.tile_pool(name="tmp", bufs=4))
        psRot = top.enter_context(
            tc.tile_pool(name="psRot", bufs=2, space="PSUM"))

        pt_s = persist.tile([P, P], F32R)
        ones_s = persist.tile([P, P], F32R)
        ident_s = persist.tile([P, P], F32R)
        maskb_s = persist.tile([P, QTW // KTW, QTW], F32)
        nc.scalar.dma_start(pt_s[:], pt_d[:].bitcast(F32R))
        nc.scalar.dma_start(ones_s[:], ones_d[:].bitcast(F32R))
        nc.scalar.dma_start(ident_s[:], ident_d[:].bitcast(F32R))
        nc.scalar.dma_start(maskb_s[:], maskb_d[:])

        # acc: [128, m, S]; m = q0..q3, k, v. fp32r so matmuls can consume it;
        # rope overwrites slots 0..4 in place, attention output overwrites
        # slots 0..3 in place. Reused across batches (WAR-tracked).
        acc = accp.tile([P, MQKV, S], F32R)
        v_nat = vnp.tile([P, NKT, HD], F32R)

        for _rep in range(reps):
          for b in range(B):
            bsl = slice(b * S, (b + 1) * S)

            cdup_b = csp.tile([P, NT, QTW], F32, tag="c")
            sdup_b = csp.tile([P, NT, QTW], F32, tag="s")
            nc.scalar.dma_start(
                cdup_b[:], cdup_d[:, bsl].rearrange("p (n q) -> p n q", q=QTW))
            nc.scalar.dma_start(
                sdup_b[:], sdup_d[:, bsl].rearrange("p (n q) -> p n q", q=QTW))

            def rope_m(m):
                """RoPE in place on acc slot m (rotation matmul + combine)."""
                for tt in range(NT):
                    tsl = slice(tt * QTW, (tt + 1) * QTW)
                    accsl = acc[:, m, tsl]
                    rps = psRot.tile([P, QTW], F32, tag="rot")
                    nc.tensor.matmul(rps[:], lhsT=pt_s[:], rhs=accsl,
                                     start=True, stop=True)
                    t1 = tmpp.tile([P, QTW], F32, tag="t1")
                    t2 = tmpp.tile([P, QTW], F32, tag="t2")
                    nc.gpsimd.tensor_mul(t1[:], accsl.bitcast(F32),
                                         cdup_b[:, tt, :])
                    nc.vector.tensor_mul(t2[:], rps[:], sdup_b[:, tt, :])
                    nc.vector.tensor_add(accsl, t1[:], t2[:])

            # ---- phase A: projections + rope(k, q0) + v transpose ----
            with ExitStack() as actx:
                xqp = actx.enter_context(tc.tile_pool(name="xq", bufs=2))
                wqp = actx.enter_context(tc.tile_pool(name="wql", bufs=2))
                psA = actx.enter_context(
                    tc.tile_pool(name="psA", bufs=2, space="PSUM"))
                psVT = actx.enter_context(
                    tc.tile_pool(name="psVT", bufs=2, space="PSUM"))

                c_off = 0
                for dq, c_n in enumerate(CSIZES):
                    dsl = slice(c_off * P, (c_off + c_n) * P)
                    c_off += c_n
                    xq = xqp.tile([P, c_n, S], F32R, tag="xq",
                                  padded_shape=[P, max(CSIZES), S])
                    xsrc = (xT_d[dsl, bsl]
                            .rearrange("(c p) t -> p c t", p=P).bitcast(F32R))
                    wql = wqp.tile([P, c_n, MQKV * P], F32R, tag="wql",
                                   padded_shape=[P, max(CSIZES), MQKV * P])
                    wsrc = (wqkvT_d[dsl, :]
                            .rearrange("(c p) m -> p c m", p=P).bitcast(F32R))
                    if dq == 0:
                        # k-head weights (cols 512:640) land first and loads
                        # are chunked so the first matmuls start early
                        nc.sync.dma_start(wql[:, :, 3 * P:], wsrc[:, :, 3 * P:])
                        nc.sync.dma_start(wql[:, :, :3 * P], wsrc[:, :, :3 * P])
                        for tt in range(NT):
                            tsl = slice(tt * QTW, (tt + 1) * QTW)
                            nc.sync.dma_start(xq[:, :, tsl], xsrc[:, :, tsl])
                    else:
                        nc.sync.dma_start(wql[:], wsrc[:])
                        nc.sync.dma_start(xq[:], xsrc[:])
                    for m in M_ORDER:
                        for tp in range(NT // 2):   # tt pairs share a psum
                            ps = psA.tile([P, 2 * QTW], F32, tag="pa")
                            for half in range(2):
                                tt = tp * 2 + half
                                for c in range(c_n):
                                    nc.tensor.matmul(
                                        ps[:, half * QTW:(half + 1) * QTW],
                                        lhsT=wql[:, c, m * P:(m + 1) * P],
                                        rhs=xq[:, c,
                                               tt * QTW:(tt + 1) * QTW],
                                        start=(c == 0), stop=(c == c_n - 1))
                            accsl = acc[:, m,
                                        tp * 2 * QTW:(tp + 1) * 2 * QTW]
                            if dq == 0:
                                nc.scalar.copy(accsl, ps[:])
                            else:
                                nc.vector.tensor_add(
                                    accsl, accsl.bitcast(F32), ps[:])
                        if dq == len(CSIZES) - 1:
                            if m in (REP, 0):
                                rope_m(m)   # k and q0; q1-3 roped in phase B
                            elif m == REP + 1:
                                for c in range(NKT):  # v -> natural layout
                                    tps = psVT.tile([P, P], F32R, tag="vt")
                                    nc.tensor.transpose(
                                        tps[:], acc[:, m, c * P:(c + 1) * P],
                                        ident_s[:])
                                    if c % 2 == 0:
                                        nc.scalar.copy(v_nat[:, c, :], tps[:])
                                    else:
                                        nc.vector.tensor_copy(
                                            v_nat[:, c, :], tps[:])

            # ---- phase B: attention (+ trailing rope), then phase C ----
            with ExitStack() as bat:
                ep = bat.enter_context(tc.tile_pool(name="e", bufs=PIPE + 6))
                rp = bat.enter_context(tc.tile_pool(name="rec", bufs=4))
                psS = bat.enter_context(
                    tc.tile_pool(name="psS", bufs=2, space="PSUM"))
                psO = bat.enter_context(
                    tc.tile_pool(name="psO", bufs=2, space="PSUM"))
                psD = bat.enter_context(
                    tc.tile_pool(name="psD", bufs=2, space="PSUM"))
                wop = bat.enter_context(tc.tile_pool(name="wo", bufs=2))
                obp = bat.enter_context(tc.tile_pool(name="ob", bufs=8))
                for h in range(REP):
                    if h + 1 < REP:
                        rope_m(h + 1)   # rope next head under this block
                    for qt in range(NT):
                        qsl = slice(qt * QTW, (qt + 1) * QTW)
                        nkt = (qt + 1) * (QTW // KTW)
                        ps_o = psO.tile([P, QTW], F32, tag="o")
                        ps_d = psD.tile([P, QTW], F32, tag="d")
                        pend = []  # [(e_tile, kt), ...]

                        def flush(upto, ps_o=ps_o, ps_d=ps_d, nkt=nkt,
                                  pend=pend):
                            while len(pend) > upto:
                                pe, pkt = pend.pop(0)
                                nc.tensor.matmul(
                                    ps_o[:], lhsT=v_nat[:, pkt, :], rhs=pe[:],
                                    start=(pkt == 0), stop=(pkt == nkt - 1),
                                    skip_group_check=True)
                                nc.tensor.matmul(
                                    ps_d[:], lhsT=ones_s[:], rhs=pe[:],
                                    start=(pkt == 0), stop=(pkt == nkt - 1),
                                    skip_group_check=True)

                        for kt in range(nkt):
                            ps_s = psS.tile([P, QTW], F32, tag="s")
                            nc.tensor.matmul(
                                ps_s[:],
                                lhsT=acc[:, REP, kt * KTW:(kt + 1) * KTW],
                                rhs=acc[:, h, qsl],
                                start=True, stop=True)
                            j = kt - qt * (QTW // KTW)
                            if j >= 0:
                                # columns q >= 128(j+1) are never masked
                                w = KTW * (j + 1)
                                nc.vector.tensor_add(
                                    ps_s[:, :w], ps_s[:, :w],
                                    maskb_s[:, j, :w])
                            e = ep.tile([P, QTW], F32R, tag="e")
                            nc.scalar.activation(
                                e[:], ps_s[:],
                                mybir.ActivationFunctionType.Exp,
                                scale=SCALE)
                            pend.append((e, kt))
                            flush(PIPE)
                        flush(0)
                        rec = rp.tile([P, QTW], F32, tag="rec")
                        nc.vector.reciprocal(rec[:], ps_d[:])
                        # overwrite q slot h with normalized attention out
                        nc.vector.tensor_mul(acc[:, h, qsl], ps_o[:], rec[:])

                # ---- phase C: output projection (partial) ----
                for nq in range(NWQ):
                    nsl = slice(nq * NQ, (nq + 1) * NQ)
                    woh = wop.tile([P, REP, NQ], F32R, tag="wo")
                    nc.scalar.dma_start(
                        woh[:],
                        woT_d[:, nsl]
                        .rearrange("(h p) n -> p h n", p=P).bitcast(F32R))
                    for tt in range(S // P):
                        ob = obp.tile([P, NQ], F32, tag="ob")
                        for half in range(NQ // QTW):
                            ps = psS.tile([P, QTW], F32, tag="s")
                            for h in range(REP):
                                nc.tensor.matmul(
                                    ps[:],
                                    lhsT=acc[:, h, tt * P:(tt + 1) * P],
                                    rhs=woh[:, h,
                                            half * QTW:(half + 1) * QTW],
                                    start=(h == 0), stop=(h == REP - 1))
                            nc.scalar.copy(
                                ob[:, half * QTW:(half + 1) * QTW], ps[:])
                        nc.sync.dma_start(
                            out_d[b * S + tt * P:b * S + (tt + 1) * P, nsl],
                            ob[:])
    nc.compile()
    return nc


def get_nc():
    global _nc
    if _nc is None:
        _nc = _build_nc()
    return _nc


def make_in_maps(x, freqs_cos, freqs_sin, wq, wk, wv, wo):
    """Host-side prep: transposes, rope tables, masks, per-core weight shards."""
    x = np.ascontiguousarray(x, np.float32)
    fc = np.asarray(freqs_cos, np.float32)
    fs = np.asarray(freqs_sin, np.float32)
    wq = np.asarray(wq, np.float32)
    wk = np.asarray(wk, np.float32)
    wv = np.asarray(wv, np.float32)
    wo = np.asarray(wo, np.float32)

    xT = np.ascontiguousarray(x.reshape(T, D).T)
    cdup = np.ascontiguousarray(np.tile(np.repeat(fc.T, 2, axis=0), (1, B)))
    sdup = np.ascontiguousarray(np.tile(np.repeat(fs.T, 2, axis=0), (1, B)))
    prot = np.zeros((P, P), np.float32)
    for i in range(P // 2):
        prot[2 * i, 2 * i + 1] = -1.0
        prot[2 * i + 1, 2 * i] = 1.0
    pt = np.ascontiguousarray(prot.T)
    ones = np.ones((P, P), np.float32)
    ident = np.eye(P, dtype=np.float32)
    ki = np.arange(KTW)[:, None]
    qi = np.arange(QTW)[None, :]
    maskb = np.stack(
        [np.where(j * KTW + ki > qi, -1e9, 0.0).astype(np.float32)
         for j in range(QTW // KTW)], axis=1)  # [128, 4, 512]
    maskb = np.ascontiguousarray(maskb)

    in_maps = []
    for g in range(NCORES):
        wq_g = wq[g * REP * HD:(g + 1) * REP * HD]
        wk_g = wk[g * HD:(g + 1) * HD]
        wv_g = wv[g * HD:(g + 1) * HD]
        wqkvT = np.ascontiguousarray(np.concatenate([wq_g, wk_g, wv_g], 0).T)
        woT = np.ascontiguousarray(wo[:, g * REP * HD:(g + 1) * REP * HD].T)
        in_maps.append({
            "xT": xT, "wqkvT": wqkvT, "woT": woT,
            "cdup": cdup, "sdup": sdup, "pt": pt, "ones": ones,
            "ident": ident, "maskb": maskb,
        })
    return in_maps


def kernel(x, freqs_cos, freqs_sin, wq, wk, wv, wo):
    from concourse.bass_utils import run_bass_kernel_spmd
    nc = get_nc()
    in_maps = make_in_maps(x, freqs_cos, freqs_sin, wq, wk, wv, wo)
    res = run_bass_kernel_spmd(nc, in_maps, core_ids=list(range(NCORES)))
    out = np.zeros((T, D), np.float64)
    for r in res.results:
        out += r["out"].astype(np.float64)
    return out.astype(np.float32).reshape(B, S, D)

